# revision 1
# baseline (speedup 1.0000x reference)
"""Causal attention on 8 TRN2 cores — key-sharded variant.

2 cores per batch; the PAIR splits the KEYS (not queries): parity c owns
key blocks S_c = sorted({2p+c} u {31-2p-c}), each core projects K^T/V for
only its 2048 keys (no duplication) but Q^T for all 4096 queries, then
computes partial unnormalized attention over its keyset for every query
block. Host combines y = (num0+num1)/(ell0+ell1) — exact, since exp uses
no max shift (scores bounded) so both cores use the same shift of 0.
"""

import math
import sys

sys.path.insert(0, "/opt/trn_rl_repo")

import ml_dtypes
import numpy as np

import concourse.mybir as mybir
import concourse.tile as tile
from concourse import bacc
from concourse.bass_utils import run_bass_kernel_spmd
from concourse.masks import make_identity

B = 4
S = 4096
D = 1024
P = 128
DC = D // P
NKB = 32             # global key blocks per batch
NQB = 32             # q blocks per core (all of them)
HALF = S // 2        # keys owned per core
BF16 = mybir.dt.bfloat16
F32 = mybir.dt.float32
NEG = -1.0e9
SCALE = 1.0 / math.sqrt(D)


def _keyset(c):
    return sorted({2 * p + c for p in range(8)} | {31 - 2 * p - c for p in range(8)})


def _L(g):
    return max((g + 2) // 2, 1)  # ceil((g+1)/2), min 1


_LS = [_L(g) for g in range(NQB)]

# attention block order: interleave big and small blocks so a big block's
# long matmul stream hides the small block's serial latency chain
_GORDER = []
_lo, _hi = 0, NQB - 1
while _lo <= _hi:
    _GORDER.append(_hi); _hi -= 1
    if _lo <= _hi:
        _GORDER.append(_lo); _lo += 1


def _build_program(reps=1):
    nc = bacc.Bacc("TRN2", target_bir_lowering=False, debug=False)

    xT = nc.dram_tensor("xT", [D, HALF], BF16, kind="ExternalInput").ap()
    xTq = nc.dram_tensor("xTq", [D, S], BF16, kind="ExternalInput").ap()
    wq = nc.dram_tensor("wq", [D, D], BF16, kind="ExternalInput").ap()
    wk = nc.dram_tensor("wk", [D, D], BF16, kind="ExternalInput").ap()
    wv = nc.dram_tensor("wv", [D, D], BF16, kind="ExternalInput").ap()
    mask = nc.dram_tensor("mask", [NQB, P, 256], F32, kind="ExternalInput").ap()
    y = nc.dram_tensor("y", [S, D], F32, kind="ExternalOutput").ap()
    ell = nc.dram_tensor("ell", [P, NQB], F32, kind="ExternalOutput").ap()

    with tile.TileContext(nc) as tc:
        with (
            tc.tile_pool(name="big", bufs=1) as big,
            tc.tile_pool(name="wpool", bufs=2) as wpool,
            tc.tile_pool(name="xslab", bufs=2) as xslab,
            tc.tile_pool(name="mk", bufs=2) as mk_pool,
            tc.tile_pool(name="pp", bufs=4) as pp_pool,
            tc.tile_pool(name="pT", bufs=3) as pT_pool,
            tc.tile_pool(name="yy", bufs=2) as y_pool,
            tc.tile_pool(name="st", bufs=2) as st_pool,
            tc.tile_pool(name="ps", bufs=6, space="PSUM") as ps,
            tc.tile_pool(name="yp", bufs=2, space="PSUM") as yp_pool,
        ):
          for _rep in range(reps):
            KT = big.tile([P, DC, HALF], BF16, tag="KT")     # 32 KB/part
            V = big.tile([P, 16, D], BF16, tag="V")          # 32 KB/part
            QT = big.tile([P, DC, S], BF16, tag="QT")        # 64 KB/part
            ells_all = big.tile([P, NQB], F32, tag="ells_all")
            ident = big.tile([P, P], BF16, tag="ident")
            make_identity(nc, ident[:])

            # ---- fused K^T + V projection over the local key half ------
            wk_t = wpool.tile([P, DC, D], BF16, tag="W")
            for _i in range(DC):
                nc.scalar.dma_start(
                    out=wk_t[:, _i, :], in_=wk[_i * P : (_i + 1) * P, :]
                )
            wv_t = wpool.tile([P, DC, D], BF16, tag="W")
            for _i in range(DC):
                nc.scalar.dma_start(
                    out=wv_t[:, _i, :], in_=wv[_i * P : (_i + 1) * P, :]
                )
            for kt in range(4):  # local key tiles of 512
                xs = xslab.tile([P, DC, 512], BF16, tag="xs")
                for _i in range(DC):
                    nc.sync.dma_start(
                        out=xs[:, _i, :],
                        in_=xT[_i * P : (_i + 1) * P,
                               kt * 512 : (kt + 1) * 512],
                    )
                for j in range(DC):
                    pt = ps.tile([P, 512], F32, tag="ps", name=f"kp{kt}_{j}")
                    for i in range(DC):
                        nc.tensor.matmul(
                            pt[:],
                            lhsT=wk_t[:, i, j * P : (j + 1) * P],
                            rhs=xs[:, i, :],
                            start=(i == 0),
                            stop=(i == DC - 1),
                        )
                    nc.vector.tensor_copy(
                        KT[:, j, kt * 512 : (kt + 1) * 512], pt[:]
                    )
                for sb in range(4):
                    kb = kt * 4 + sb
                    pv = [ps.tile([P, 512], F32, tag="ps", name=f"v{n}_{kb}")
                          for n in range(2)]
                    for i in range(DC):
                        for n in range(2):
                            nc.tensor.matmul(
                                pv[n][:],
                                lhsT=xs[:, i, sb * P : (sb + 1) * P],
                                rhs=wv_t[:, i, n * 512 : (n + 1) * 512],
                                start=(i == 0),
                                stop=(i == DC - 1),
                            )
                    for n in range(2):
                        nc.scalar.copy(V[:, kb, n * 512 : (n + 1) * 512], pv[n][:])

            # ---- Q^T projection for ALL queries, SBUF-resident ---------
            wq_t = wpool.tile([P, DC, D], BF16, tag="W")
            for _i in range(DC):
                nc.scalar.dma_start(
                    out=wq_t[:, _i, :], in_=wq[_i * P : (_i + 1) * P, :]
                )
            for qt in range(8):  # q tiles of 512
                xs = xslab.tile([P, DC, 512], BF16, tag="xs")
                for _i in range(DC):
                    nc.sync.dma_start(
                        out=xs[:, _i, :],
                        in_=xTq[_i * P : (_i + 1) * P,
                                qt * 512 : (qt + 1) * 512],
                    )
                for j in range(DC):
                    pt = ps.tile([P, 512], F32, tag="ps", name=f"qp{qt}_{j}")
                    for i in range(DC):
                        nc.tensor.matmul(
                            pt[:],
                            lhsT=wq_t[:, i, j * P : (j + 1) * P],
                            rhs=xs[:, i, :],
                            start=(i == 0),
                            stop=(i == DC - 1),
                        )
                    nc.vector.tensor_copy(
                        QT[:, j, qt * 512 : (qt + 1) * 512], pt[:]
                    )

            # ---- partial causal attention over the local keyset --------
            for g in _GORDER:
                L = _LS[g]
                cols = L * P
                T = (cols + 511) // 512
                widths = [512] * (T - 1) + [cols - 512 * (T - 1)]
                mw = 128 if L == 1 else 256

                mk = mk_pool.tile([P, 256], F32, tag="mk")
                nc.sync.dma_start(out=mk[:], in_=mask[g])

                pts = []
                for t in range(T):
                    pts.append(ps.tile([P, widths[t]], F32, tag="ps",
                                       name=f"sc{g}_{t}"))
                for i in range(DC):
                    for t in range(T):
                        nc.tensor.matmul(
                            pts[t][:],
                            lhsT=QT[:, i, g * P : (g + 1) * P],
                            rhs=KT[:, i, t * 512 : t * 512 + widths[t]],
                            start=(i == 0),
                            stop=(i == DC - 1),
                        )

                # additive mask on the last mw local kv columns (the
                # window can straddle the last two PSUM tiles)
                rem = mw
                moff = mw
                ti = T - 1
                while rem > 0:
                    w = widths[ti]
                    take = min(rem, w)
                    nc.vector.tensor_add(
                        pts[ti][:, w - take : w],
                        pts[ti][:, w - take : w],
                        mk[:, moff - take : moff],
                    )
                    rem -= take
                    moff -= take
                    ti -= 1

                ells = st_pool.tile([P, 8], F32, tag="ells")
                yps = [yp_pool.tile([P, 512], F32, tag="yp", name=f"y{n}_{g}")
                       for n in range(2)]

                def attnv(m, psb):
                    for n in range(2):
                        nc.tensor.matmul(
                            yps[n][:],
                            lhsT=psb[:],
                            rhs=V[:, m, n * 512 : (n + 1) * 512],
                            start=(m == 0),
                            stop=(m == L - 1),
                        )

                kc = 0
                pending = None
                for t in range(T):
                    ppt = pp_pool.tile([P, widths[t]], BF16, tag="pp",
                                       name=f"pp{g}_{t}")
                    nc.scalar.activation(
                        ppt[:],
                        pts[t][:],
                        mybir.ActivationFunctionType.Exp,
                        bias=0.0,
                        scale=SCALE,
                        accum_out=ells[:, t : t + 1],
                    )
                    for cch in range(widths[t] // P):
                        ptp = ps.tile([P, P], BF16, tag="ps", name=f"tp{g}_{kc}")
                        nc.tensor.transpose(
                            ptp[:], ppt[:, cch * P : (cch + 1) * P], ident[:]
                        )
                        psb = pT_pool.tile([P, P], BF16, tag="pT",
                                           name=f"pb{g}_{kc}")
                        nc.vector.tensor_copy(psb[:], ptp[:])
                        if pending is not None:
                            attnv(*pending)
                        pending = (kc, psb)
                        kc += 1
                attnv(*pending)

                nc.vector.tensor_reduce(
                    ells_all[:, g : g + 1],
                    ells[:, :T],
                    axis=mybir.AxisListType.X,
                    op=mybir.AluOpType.add,
                )

                for n in range(2):  # unnormalized numerator out
                    ys = y_pool.tile([P, 512], F32, tag="y")
                    nc.scalar.copy(ys[:], yps[n][:])
                    nc.sync.dma_start(
                        out=y[g * P : (g + 1) * P, n * 512 : (n + 1) * 512],
                        in_=ys[:],
                    )

            nc.sync.dma_start(out=ell[:, :], in_=ells_all[:])
    nc.finalize()
    return nc


_NC = None


def _get_program():
    global _NC
    if _NC is None:
        _NC = _build_program()
    return _NC


def _build_mask(c):
    """mask[g, :, j] (j < mw) applies to local kv col L*128 - mw + j."""
    ks = _keyset(c)
    m = np.full((NQB, P, 256), NEG, np.float32)
    for g in range(NQB):
        L = _LS[g]
        cnt = sum(1 for b in ks if b <= g)
        mw = 128 if L == 1 else 256
        q = g * P + np.arange(P)[:, None]
        for wi in range(mw // P):
            mlocal = L - mw // P + wi
            if mlocal < cnt:
                gb = ks[mlocal]
                k = gb * P + np.arange(P)[None, :]
                m[g, :, wi * P : (wi + 1) * P] = np.where(k <= q, 0.0, NEG)
    return m


def kernel(x, Wq, Wk, Wv):
    bf = ml_dtypes.bfloat16
    nc = _get_program()

    wqb = np.ascontiguousarray(Wq.astype(bf))
    wkb = np.ascontiguousarray(Wk.astype(bf))
    wvb = np.ascontiguousarray(Wv.astype(bf))
    masks = [_build_mask(0), _build_mask(1)]
    keycols = [
        np.concatenate([np.arange(b * P, (b + 1) * P) for b in _keyset(c)])
        for c in (0, 1)
    ]

    in_maps = []
    for core in range(8):
        b, c = core // 2, core % 2
        xb = x[b]
        in_maps.append(
            {
                "xT": np.ascontiguousarray(xb[keycols[c]].T.astype(bf)),
                "xTq": np.ascontiguousarray(xb.T.astype(bf)),
                "wq": wqb,
                "wk": wkb,
                "wv": wvb,
                "mask": masks[c],
            }
        )

    res = run_bass_kernel_spmd(nc, in_maps, core_ids=list(range(8))).results

    out = np.empty((B, S, D), np.float32)
    for b in range(B):
        r0, r1 = res[2 * b], res[2 * b + 1]
        num = r0["y"] + r1["y"]
        # ell[p, g] -> per-row: q = g*128 + p
        l0 = r0["ell"].T.reshape(S, 1)
        l1 = r1["ell"].T.reshape(S, 1)
        out[b] = num / (l0 + l1)
    return out



# revision 2
# speedup vs baseline: 1.1979x; 1.1979x over previous
"""Causal attention on 8 TRN2 cores — transposed-scores variant (v2).

2 cores per batch; the PAIR splits the KEYS: parity c owns key blocks
S_c = sorted({2p+c} u {31-2p-c}) (16 of 32 blocks), projects K^T/V for
its 2048 keys + Q^T for all 4096 queries, computes partial unnormalized
attention, host combines y = (num0+num1)/(ell0+ell1).

v2 computes scores TRANSPOSED: S_T[k, q] = (KT_i-block stationary) @
(QT_i-chunk moving), so exp(S_T) is directly the stationary operand of
the attn@V matmul — no PE transposes, no PSUM->SBUF prob copies, and
ell comes from tiny P_T @ ones matmuls. Queries are processed in chunks
of 4 blocks (512 q) so the scores matmul keeps N=512; Q projection is
interleaved chunk-by-chunk so only 512 q-columns of QT live in SBUF.

Program structure is parity-independent (one SPMD NEFF); parity enters
only through data: which keys are in xT, and the 2 diagonal-band masks
per chunk (keyset blocks 2c and 2c+1) that cut beyond-causal entries.
"""

import math
import sys

sys.path.insert(0, "/opt/trn_rl_repo")

import ml_dtypes
import numpy as np

import concourse.mybir as mybir
import concourse.tile as tile
from concourse import bacc
from concourse.bass_utils import run_bass_kernel_spmd

B = 4
S = 4096
D = 1024
P = 128
DC = D // P          # 8 chunks of the contraction dim
NQB = 32             # query blocks per batch
NCH = 8              # query chunks (4 blocks = 512 q each)
HALF = S // 2        # keys owned per core
BF16 = mybir.dt.bfloat16
F32 = mybir.dt.float32
NEG = -1.0e9
SCALE = 1.0 / math.sqrt(D)


def _keyset(c):
    return sorted({2 * p + c for p in range(8)} | {31 - 2 * p - c for p in range(8)})


def _L(g):
    return (g + 2) // 2  # ceil((g+1)/2): unified per-parity kv-block count


def _build_program(reps=1):
    nc = bacc.Bacc("TRN2", target_bir_lowering=False, debug=False)

    xT = nc.dram_tensor("xT", [D, HALF], BF16, kind="ExternalInput").ap()
    xTq = nc.dram_tensor("xTq", [D, S], BF16, kind="ExternalInput").ap()
    wq = nc.dram_tensor("wq", [D, D], BF16, kind="ExternalInput").ap()
    wk = nc.dram_tensor("wk", [D, D], BF16, kind="ExternalInput").ap()
    wv = nc.dram_tensor("wv", [D, D], BF16, kind="ExternalInput").ap()
    mask = nc.dram_tensor("mask", [2 * NCH, P, 512], F32, kind="ExternalInput").ap()
    y = nc.dram_tensor("y", [S, D], BF16, kind="ExternalOutput").ap()
    ell = nc.dram_tensor("ell", [1, S], F32, kind="ExternalOutput").ap()

    with tile.TileContext(nc) as tc:
        with (
            tc.tile_pool(name="big", bufs=1) as big,
            tc.tile_pool(name="wpool", bufs=2) as wpool,
            tc.tile_pool(name="wqp", bufs=1) as wqp,
            tc.tile_pool(name="xslab", bufs=2) as xslab,
            tc.tile_pool(name="qt", bufs=2) as qt_pool,
            tc.tile_pool(name="mk", bufs=2) as mk_pool,
            tc.tile_pool(name="pT", bufs=2) as pT_pool,
            tc.tile_pool(name="yy", bufs=2) as y_pool,
            tc.tile_pool(name="els", bufs=2) as els_pool,
            tc.tile_pool(name="ps", bufs=3, space="PSUM") as ps,
            tc.tile_pool(name="yp", bufs=4, space="PSUM") as yp_pool,
            tc.tile_pool(name="elp", bufs=1, space="PSUM") as el_pool,
        ):
          for _rep in range(reps):
            KT = big.tile([P, DC, HALF], BF16, tag="KT")     # 32 KB/part
            V = big.tile([P, 16, D], BF16, tag="V")          # 32 KB/part
            ones = big.tile([P, 1], BF16, tag="ones")
            nc.gpsimd.memset(ones[:], 1.0)

            # ---- fused K^T + V projection over the local key half ------
            wk_t = wpool.tile([P, DC, D], BF16, tag="W")
            for _i in range(DC):
                nc.scalar.dma_start(
                    out=wk_t[:, _i, :], in_=wk[_i * P : (_i + 1) * P, :]
                )
            wv_t = wpool.tile([P, DC, D], BF16, tag="W")
            for _i in range(DC):
                nc.scalar.dma_start(
                    out=wv_t[:, _i, :], in_=wv[_i * P : (_i + 1) * P, :]
                )
            for kt in range(4):  # local key tiles of 512
                xs = xslab.tile([P, DC, 512], BF16, tag="xs")
                for _i in range(DC):
                    nc.sync.dma_start(
                        out=xs[:, _i, :],
                        in_=xT[_i * P : (_i + 1) * P,
                               kt * 512 : (kt + 1) * 512],
                    )
                for j in range(DC):
                    pt = ps.tile([P, 512], F32, tag="ps", name=f"kp{kt}_{j}")
                    for i in range(DC):
                        nc.tensor.matmul(
                            pt[:],
                            lhsT=wk_t[:, i, j * P : (j + 1) * P],
                            rhs=xs[:, i, :],
                            start=(i == 0),
                            stop=(i == DC - 1),
                        )
                    nc.vector.tensor_copy(
                        KT[:, j, kt * 512 : (kt + 1) * 512], pt[:]
                    )
                for sb in range(4):
                    kb = kt * 4 + sb
                    pv = [ps.tile([P, 512], F32, tag="ps", name=f"v{n}_{kb}")
                          for n in range(2)]
                    for i in range(DC):
                        for n in range(2):
                            nc.tensor.matmul(
                                pv[n][:],
                                lhsT=xs[:, i, sb * P : (sb + 1) * P],
                                rhs=wv_t[:, i, n * 512 : (n + 1) * 512],
                                start=(i == 0),
                                stop=(i == DC - 1),
                            )
                    for n in range(2):
                        nc.scalar.copy(V[:, kb, n * 512 : (n + 1) * 512], pv[n][:])

            wq_t = wqp.tile([P, DC, D], BF16, tag="Wq")
            for _i in range(DC):
                nc.scalar.dma_start(
                    out=wq_t[:, _i, :], in_=wq[_i * P : (_i + 1) * P, :]
                )

            # ---- per query chunk: Q^T proj, transposed scores, attn@V --
            for c in range(NCH):
                Lmax = 2 * c + 2

                # Q^T projection for this chunk's 512 queries
                xs = xslab.tile([P, DC, 512], BF16, tag="xs")
                for _i in range(DC):
                    nc.sync.dma_start(
                        out=xs[:, _i, :],
                        in_=xTq[_i * P : (_i + 1) * P,
                                c * 512 : (c + 1) * 512],
                    )
                QT = qt_pool.tile([P, DC, 512], BF16, tag="QT")
                for j in range(DC):
                    pt = ps.tile([P, 512], F32, tag="ps", name=f"qp{c}_{j}")
                    for i in range(DC):
                        nc.tensor.matmul(
                            pt[:],
                            lhsT=wq_t[:, i, j * P : (j + 1) * P],
                            rhs=xs[:, i, :],
                            start=(i == 0),
                            stop=(i == DC - 1),
                        )
                    nc.vector.tensor_copy(QT[:, j, :], pt[:])

                # masks for the two diagonal-band key blocks (kbi 2c, 2c+1)
                mks = []
                for sl in range(2):
                    mk = mk_pool.tile([P, 512], F32, tag="mk",
                                      name=f"mk{c}_{sl}")
                    nc.sync.dma_start(out=mk[:], in_=mask[2 * c + sl])
                    mks.append(mk)

                # transposed scores S_T[k, q] + exp -> P_T, per local kv blk.
                # ell[q] = sum_k P_T[k, q] accumulates via a ones-stationary
                # matmul per block, lagged one block behind the scores so the
                # PE never waits on the exp that produces its rhs.
                pT = pT_pool.tile([P, 16, 512], BF16, tag="pT")
                elps = el_pool.tile([1, 512], F32, tag="elp", name=f"elp{c}")

                def ell_mm(kbi):
                    nc.tensor.matmul(
                        elps[0:1, :],
                        lhsT=ones[:, 0:1],
                        rhs=pT[:, kbi, :],
                        start=(kbi == 0),
                        stop=(kbi == Lmax - 1),
                    )

                for kbi in range(Lmax):
                    pts = ps.tile([P, 512], F32, tag="ps", name=f"sc{c}_{kbi}")
                    for i in range(DC):
                        nc.tensor.matmul(
                            pts[:],
                            lhsT=KT[:, i, kbi * P : (kbi + 1) * P],
                            rhs=QT[:, i, :],
                            start=(i == 0),
                            stop=(i == DC - 1),
                        )
                    if kbi >= 2 * c:
                        nc.vector.tensor_add(pts[:], pts[:], mks[kbi - 2 * c][:])
                    nc.scalar.activation(
                        pT[:, kbi, :],
                        pts[:],
                        mybir.ActivationFunctionType.Exp,
                        bias=0.0,
                        scale=SCALE,
                    )
                    if kbi > 0:
                        ell_mm(kbi - 1)

                # attn @ V, per query block of the chunk
                for gi in range(4):
                    g = 4 * c + gi
                    Lg = _L(g)
                    yps = [yp_pool.tile([P, 512], F32, tag="yp",
                                        name=f"y{n}_{g}")
                           for n in range(2)]
                    for kbi in range(Lg):
                        pslab = pT[:, kbi, gi * P : (gi + 1) * P]
                        for n in range(2):
                            nc.tensor.matmul(
                                yps[n][:],
                                lhsT=pslab,
                                rhs=V[:, kbi, n * 512 : (n + 1) * 512],
                                start=(kbi == 0),
                                stop=(kbi == Lg - 1),
                            )
                    if gi == 0:
                        ell_mm(Lmax - 1)
                        els = els_pool.tile([1, 512], F32, tag="els")
                        nc.vector.tensor_copy(els[0:1, :], elps[0:1, :])
                        nc.sync.dma_start(
                            out=ell[0:1, c * 512 : (c + 1) * 512], in_=els[0:1, :]
                        )
                    # evacuate numerator: n=0 on scalar, n=1 on vector so
                    # neither engine bottlenecks the early (small-L) chunks
                    ys0 = y_pool.tile([P, 512], BF16, tag="y")
                    nc.scalar.copy(ys0[:], yps[0][:])
                    nc.sync.dma_start(
                        out=y[g * P : (g + 1) * P, 0:512], in_=ys0[:]
                    )
                    ys1 = y_pool.tile([P, 512], BF16, tag="y")
                    nc.vector.tensor_copy(ys1[:], yps[1][:])
                    nc.sync.dma_start(
                        out=y[g * P : (g + 1) * P, 512:1024], in_=ys1[:]
                    )
    nc.finalize()
    return nc


_NC = None


def _get_program():
    global _NC
    if _NC is None:
        _NC = _build_program()
    return _NC


def _build_mask(c):
    """mask[2*ch+sl, p, qcol]: additive mask for key block ks[2*ch+sl]
    against query chunk ch (global q = ch*512 + qcol, k = b*128 + p)."""
    ks = _keyset(c)
    m = np.zeros((2 * NCH, P, 512), np.float32)
    q = np.arange(512)[None, :]
    p = np.arange(P)[:, None]
    for ch in range(NCH):
        for sl in range(2):
            b = ks[2 * ch + sl]
            keep = (b * P + p) <= (ch * 512 + q)
            m[2 * ch + sl] = np.where(keep, 0.0, NEG)
    return m


def kernel(x, Wq, Wk, Wv):
    bf = ml_dtypes.bfloat16
    nc = _get_program()

    wqb = np.ascontiguousarray(Wq.astype(bf))
    wkb = np.ascontiguousarray(Wk.astype(bf))
    wvb = np.ascontiguousarray(Wv.astype(bf))
    masks = [_build_mask(0), _build_mask(1)]
    keycols = [
        np.concatenate([np.arange(b * P, (b + 1) * P) for b in _keyset(c)])
        for c in (0, 1)
    ]

    in_maps = []
    for core in range(8):
        b, c = core // 2, core % 2
        xb = x[b]
        in_maps.append(
            {
                "xT": np.ascontiguousarray(xb[keycols[c]].T.astype(bf)),
                "xTq": np.ascontiguousarray(xb.T.astype(bf)),
                "wq": wqb,
                "wk": wkb,
                "wv": wvb,
                "mask": masks[c],
            }
        )

    res = run_bass_kernel_spmd(nc, in_maps, core_ids=list(range(8))).results

    out = np.empty((B, S, D), np.float32)
    for b in range(B):
        r0, r1 = res[2 * b], res[2 * b + 1]
        num = r0["y"].astype(np.float32) + r1["y"].astype(np.float32)
        l0 = r0["ell"].reshape(S, 1)
        l1 = r1["ell"].reshape(S, 1)
        out[b] = num / (l0 + l1)
    return out
